# revision 18
# baseline (speedup 1.0000x reference)
"""GaussianBlur2D (11x11, reflect pad) on 8 Trainium2 NeuronCores via Bass/Tile.

Math: the 2D Gaussian is separable, and 1D conv with reflect padding over a
length-512 axis is a banded 512x512 matrix A (bandwidth 11).  So per (B,C)
plane:  Y = A @ X @ A.T.

Kernel structure (per core, 128 planes):
  pass 1:  W1T = X^T A^T    out[w, i] = sum_h X[h, w] * A^T[h, i]
  pass 2:  Y   = W1 A^T     out[i, j] = sum_w W1T[w, i] * A^T[w, j]
Both passes use the *data* 128x128 tile as the stationary operand and A^T as
the moving operand, so no transposes are ever needed and the output lands in
natural layout.  The band |h-i|<=5 restricts each contraction block's moving
window to <=138 of 512 columns; PSUM has_written semantics (start=True clears
the bank, start=False overwrites-where-unwritten / accumulates-where-written)
let the 4 chain-overlapping windows compose with no seam fixups.

Data-parallel over 8 cores: 1024 (B*C) planes -> 128 per core.  bf16 in/out
(rel-err budget 2e-2; bf16 path measures ~1e-3), fp32 PSUM accumulation.
"""

import sys

import numpy as np

sys.path.insert(0, "/opt/trn_rl_repo")

import ml_dtypes  # noqa: E402

KS = 11
PAD = (KS - 1) // 2
H = W = 512
B, C = 16, 64
N_CORES = 8
PLANES = (B * C) // N_CORES  # 128 planes per core
NB = H // 128  # 4 partition blocks per axis

# "overlap": 4 chain-overlapping banded matmuls per output block; relies on
#   per-element PSUM has_written (overwrite-where-unwritten) — fastest.
# "seam": non-overlapping main windows + 10-col accumulate-only seam matmuls;
#   each seam is a subset of the immediately preceding main window, so it is
#   correct even under bank-granular has_written clears (and in CoreSim).
MODE = "overlap"

_cached = {}


def _conv_matrix(g1d: np.ndarray, n: int) -> np.ndarray:
    """Banded matrix A s.t. (A @ v) = 1D conv of v with g1d, reflect pad."""
    k = g1d.shape[0]
    pad = (k - 1) // 2
    idx = np.arange(-pad, n + pad)
    idx = np.abs(idx)  # reflect at 0
    idx = np.where(idx >= n, 2 * (n - 1) - idx, idx)  # reflect at n-1
    A = np.zeros((n, n), dtype=np.float64)
    for i in range(n):
        for t in range(k):
            A[i, idx[i + t]] += g1d[t]
    return A


def _windows():
    # moving-operand / psum column window [lo, hi) per contraction block k
    return [
        (max(0, 128 * k - PAD), min(H, 128 * k + 128 + PAD)) for k in range(NB)
    ]


def _build():
    from concourse import bacc, mybir
    from concourse.tile import TileContext

    bf16 = mybir.dt.bfloat16
    f32 = mybir.dt.float32

    # Bacc (not raw Bass): its compile() runs move_matmul_waits_to_ldweights
    # + generate_event_semaphores, which split multi-wait sync into event-sem
    # instructions — walrus only accepts 1 wait per instruction.
    nc = bacc.Bacc(None, target_bir_lowering=False)
    # DRAM layouts are partition-major ([p][k][w]) so every plane load/store
    # is one fully contiguous 512 KiB DMA with 4 KiB per partition; the host
    # folds the (p,k) permutation into the fp32<->bf16 conversion pass.
    x_d = nc.declare_dram_parameter("x", [PLANES, 128, NB, W], bf16, isOutput=False)
    at_d = nc.declare_dram_parameter("at", [128, NB, H], bf16, isOutput=False)
    y_d = nc.declare_dram_parameter("y", [PLANES, 128, NB, W], bf16, isOutput=True)

    wins = _windows()

    with TileContext(nc) as tc:
        with (
            tc.tile_pool(name="const", bufs=1) as cpool,
            # bufs=8: slot-reuse distance (in DMA issue order) must be ≡0 mod 8
            # so WAW partners share a DMA sem lane (FIFO ⇒ no extra wait; the
            # HWDGE descriptor only fits one wait + one update)
            tc.tile_pool(name="xp", bufs=8) as xpool,
            tc.tile_pool(name="mp", bufs=3) as mpool,
            tc.tile_pool(name="yp", bufs=4) as ypool,
            tc.tile_pool(name="ps", bufs=8, space="PSUM") as pspool,
        ):
            at_s = cpool.tile([128, NB, H], bf16)
            nc.sync.dma_start(out=at_s[:], in_=at_d[:])

            def blur_pass(src, dst, pass_idx):
                # src/dst: [128, NB, 512] bf16; per output block m emit the
                # banded matmuls into one PSUM bank, then cast-copy to SBUF.
                for m in range(NB):
                    ps = pspool.tile([128, W], f32, tag="ps")
                    msl = slice(128 * m, 128 * (m + 1))
                    if MODE == "overlap":
                        for k in range(NB):
                            lo, hi = wins[k]
                            nc.tensor.matmul(
                                ps[:, lo:hi],
                                src[:, k, msl],
                                at_s[:, k, lo:hi],
                                start=(k == 0),
                                stop=(k == NB - 1),
                            )
                    else:  # seam mode
                        for k in range(NB):
                            lo = 0 if k == 0 else 128 * k - PAD
                            hi = H if k == NB - 1 else 128 * k + 128 - PAD
                            nc.tensor.matmul(
                                ps[:, lo:hi],
                                src[:, k, msl],
                                at_s[:, k, lo:hi],
                                start=True,
                                stop=(k == 0),
                            )
                            if k >= 1:
                                slo, shi = 128 * k - PAD, 128 * k + PAD
                                nc.tensor.matmul(
                                    ps[:, slo:shi],
                                    src[:, k - 1, msl],
                                    at_s[:, k - 1, slo:shi],
                                    start=False,
                                    stop=True,
                                )
                    if (m + pass_idx) % 2 == 0:
                        nc.vector.tensor_copy(out=dst[:, m, :], in_=ps[:])
                    else:
                        nc.scalar.copy(out=dst[:, m, :], in_=ps[:])

            mids = {}
            for p in range(PLANES + 1):
                if p < PLANES:
                    xt = xpool.tile([128, NB, W], bf16, tag="x")
                    nc.sync.dma_start(out=xt[:], in_=x_d[p])
                    mid = mpool.tile([128, NB, H], bf16, tag="mid")
                    blur_pass(xt, mid, 0)
                    mids[p] = mid
                if p >= 1:
                    q = p - 1
                    yst = ypool.tile([128, NB, W], bf16, tag="yst")
                    blur_pass(mids.pop(q), yst, 1)
                    nc.sync.dma_start(out=y_d[q], in_=yst[:])

    _strip_self_waits(nc)
    nc.compile()
    return nc


def _strip_self_waits(nc):
    """Drop waits on an instruction's own engine semaphore.

    PE/ACT/DVE execute and complete their streams in order, so a wait on the
    engine's own proc sem is already enforced by program order.  Tile's
    vector clock still emits them (it is not transitively minimal), and the
    walrus MATMULT struct only fits one wait + one update — the redundant
    self-wait breaks codegen.  DMA lane FIFO is NOT guaranteed across HW
    queues and GpSimd runs 8 Q7s concurrently, so only PE/ACT/DVE non-DMA
    instructions are touched.
    """
    own = {"PE": "PE_", "Activation": "Activation_", "DVE": "DVE_"}
    for blk in nc.m.functions[0].blocks:
        for inst in blk.instructions:
            eng = getattr(inst, "engine", None)
            pref = own.get(getattr(eng, "name", None))
            if pref is None or "DMA" in inst.__class__.__name__:
                continue
            si = getattr(inst, "sync_info", None)
            if si and si.on_wait:
                kept = [
                    w
                    for w in si.on_wait
                    if not (getattr(w, "ant_name", "") or "").startswith(pref)
                ]
                if len(kept) != len(si.on_wait):
                    si.on_wait = kept


def _get_nc():
    if "nc" not in _cached:
        _cached["nc"] = _build()
    return _cached["nc"]


def _prep_inputs(x: np.ndarray, sigma: np.ndarray):
    s = abs(float(np.asarray(sigma).reshape(-1)[0])) + 1e-6
    r = np.arange(KS, dtype=np.float64) - (KS - 1) / 2.0
    g = np.exp(-(r**2) / (2.0 * s**2))
    g = g / g.sum()  # normalized 1D; outer(g, g) == reference 2D kernel
    A = _conv_matrix(g, H)  # (512, 512) float64, exact
    bf = ml_dtypes.bfloat16
    # device layouts are partition-major: [p][k][...] with h = 128*k + p
    at = A.T.reshape(NB, 128, H).transpose(1, 0, 2).astype(bf)

    xb = np.asarray(x, dtype=np.float32).reshape(N_CORES, PLANES, NB, 128, W)
    xb = xb.transpose(0, 1, 3, 2, 4).astype(bf)  # permute + cast in one pass
    return xb, at


def run(x: np.ndarray, sigma: np.ndarray, trace: bool = False):
    """Run on HW; returns (output fp32 (B,C,H,W), BassKernelResults)."""
    from concourse.bass_utils import run_bass_kernel_spmd

    nc = _get_nc()
    xb, at = _prep_inputs(x, sigma)
    in_maps = [{"x": xb[i], "at": at} for i in range(N_CORES)]
    res = run_bass_kernel_spmd(nc, in_maps, list(range(N_CORES)), trace=trace)
    out = np.empty((N_CORES, PLANES, H, W), dtype=np.float32)
    for i in range(N_CORES):
        yd = res.results[i]["y"]  # [PLANES, 128, NB, W], i_row = 128*k + p
        out[i] = yd.transpose(0, 2, 1, 3).reshape(PLANES, H, W).astype(np.float32)
    return out.reshape(B, C, H, W), res


def kernel(x: np.ndarray, sigma: np.ndarray) -> np.ndarray:
    out, _ = run(x, sigma, trace=False)
    return out


# revision 20
# speedup vs baseline: 1.1591x; 1.1591x over previous
"""GaussianBlur2D (11x11, reflect pad) on 8 Trainium2 NeuronCores via Bass/Tile.

Math: the 2D Gaussian is separable, and 1D conv with reflect padding over a
length-512 axis is a banded 512x512 matrix A (bandwidth 11).  So per (B,C)
plane:  Y = A @ X @ A.T.

Kernel structure (per core, 128 planes):
  pass 1:  W1T = X^T A^T    out[w, i] = sum_h X[h, w] * A^T[h, i]
  pass 2:  Y   = W1 A^T     out[i, j] = sum_w W1T[w, i] * A^T[w, j]
Both passes use the *data* 128x128 tile as the stationary operand and A^T as
the moving operand, so no transposes are ever needed and the output lands in
natural layout.  The band |h-i|<=5 restricts each contraction block's moving
window to <=138 of 512 columns; PSUM has_written semantics (start=True clears
the bank, start=False overwrites-where-unwritten / accumulates-where-written)
let the 4 chain-overlapping windows compose with no seam fixups.

Data-parallel over 8 cores: 1024 (B*C) planes -> 128 per core.  bf16 in/out
(rel-err budget 2e-2; bf16 path measures ~1e-3), fp32 PSUM accumulation.
"""

import sys

import numpy as np

sys.path.insert(0, "/opt/trn_rl_repo")

import ml_dtypes  # noqa: E402

KS = 11
PAD = (KS - 1) // 2
H = W = 512
B, C = 16, 64
N_CORES = 8
PLANES = (B * C) // N_CORES  # 128 planes per core
NB = H // 128  # 4 partition blocks per axis

# "overlap": 4 chain-overlapping banded matmuls per output block; relies on
#   per-element PSUM has_written (overwrite-where-unwritten) — fastest.
# "seam": non-overlapping main windows + 10-col accumulate-only seam matmuls;
#   each seam is a subset of the immediately preceding main window, so it is
#   correct even under bank-granular has_written clears (and in CoreSim).
MODE = "overlap"

_cached = {}


def _conv_matrix(g1d: np.ndarray, n: int) -> np.ndarray:
    """Banded matrix A s.t. (A @ v) = 1D conv of v with g1d, reflect pad."""
    k = g1d.shape[0]
    pad = (k - 1) // 2
    idx = np.arange(-pad, n + pad)
    idx = np.abs(idx)  # reflect at 0
    idx = np.where(idx >= n, 2 * (n - 1) - idx, idx)  # reflect at n-1
    A = np.zeros((n, n), dtype=np.float64)
    for i in range(n):
        for t in range(k):
            A[i, idx[i + t]] += g1d[t]
    return A


def _windows():
    # moving-operand / psum column window [lo, hi) per contraction block k
    return [
        (max(0, 128 * k - PAD), min(H, 128 * k + 128 + PAD)) for k in range(NB)
    ]


def _build():
    from concourse import bacc, mybir
    from concourse.tile import TileContext

    bf16 = mybir.dt.bfloat16
    f32 = mybir.dt.float32

    # Bacc (not raw Bass): its compile() runs move_matmul_waits_to_ldweights
    # + generate_event_semaphores, which split multi-wait sync into event-sem
    # instructions — walrus only accepts 1 wait per instruction.
    nc = bacc.Bacc(None, target_bir_lowering=False)
    # DRAM layouts are partition-major ([p][k][w]) so every plane load/store
    # is one fully contiguous 512 KiB DMA with 4 KiB per partition; the host
    # folds the (p,k) permutation into the fp32<->bf16 conversion pass.
    x_d = nc.declare_dram_parameter("x", [PLANES, 128, NB, W], bf16, isOutput=False)
    at_d = nc.declare_dram_parameter("at", [128, NB, H], bf16, isOutput=False)
    y_d = nc.declare_dram_parameter("y", [PLANES, 128, NB, W], bf16, isOutput=True)

    wins = _windows()

    with TileContext(nc) as tc:
        with (
            tc.tile_pool(name="const", bufs=1) as cpool,
            # bufs=8: slot-reuse distance (in DMA issue order) must be ≡0 mod 8
            # so WAW partners share a DMA sem lane (FIFO ⇒ no extra wait; the
            # HWDGE descriptor only fits one wait + one update)
            tc.tile_pool(name="xp", bufs=8) as xpool,
            tc.tile_pool(name="mp", bufs=3) as mpool,
            tc.tile_pool(name="yp", bufs=4) as ypool,
            tc.tile_pool(name="ps", bufs=8, space="PSUM") as pspool,
        ):
            at_s = cpool.tile([128, NB, H], bf16)
            nc.gpsimd.dma_start(out=at_s[:], in_=at_d[:])

            def blur_pass(src, dst, pass_idx):
                # src/dst: [128, NB, 512] bf16; per output block m emit the
                # banded matmuls into one PSUM bank, then cast-copy to SBUF.
                for m in range(NB):
                    ps = pspool.tile([128, W], f32, tag="ps")
                    msl = slice(128 * m, 128 * (m + 1))
                    if MODE == "overlap":
                        for k in range(NB):
                            lo, hi = wins[k]
                            nc.tensor.matmul(
                                ps[:, lo:hi],
                                src[:, k, msl],
                                at_s[:, k, lo:hi],
                                start=(k == 0),
                                stop=(k == NB - 1),
                            )
                    else:  # seam mode
                        for k in range(NB):
                            lo = 0 if k == 0 else 128 * k - PAD
                            hi = H if k == NB - 1 else 128 * k + 128 - PAD
                            nc.tensor.matmul(
                                ps[:, lo:hi],
                                src[:, k, msl],
                                at_s[:, k, lo:hi],
                                start=True,
                                stop=(k == 0),
                            )
                            if k >= 1:
                                slo, shi = 128 * k - PAD, 128 * k + PAD
                                nc.tensor.matmul(
                                    ps[:, slo:shi],
                                    src[:, k - 1, msl],
                                    at_s[:, k - 1, slo:shi],
                                    start=False,
                                    stop=True,
                                )
                    if (m + pass_idx) % 2 == 0:
                        nc.vector.tensor_copy(out=dst[:, m, :], in_=ps[:])
                    else:
                        nc.scalar.copy(out=dst[:, m, :], in_=ps[:])

            mids = {}
            for p in range(PLANES + 1):
                if p < PLANES:
                    xt = xpool.tile([128, NB, W], bf16, tag="x")
                    # loads ride SWDGE (gpsimd), stores HWDGE (sync): two
                    # independent DMA issue paths share the 16 engines better
                    nc.gpsimd.dma_start(out=xt[:], in_=x_d[p])
                    mid = mpool.tile([128, NB, H], bf16, tag="mid")
                    blur_pass(xt, mid, 0)
                    mids[p] = mid
                if p >= 1:
                    q = p - 1
                    yst = ypool.tile([128, NB, W], bf16, tag="yst")
                    blur_pass(mids.pop(q), yst, 1)
                    nc.sync.dma_start(out=y_d[q], in_=yst[:])

    _strip_self_waits(nc)
    nc.compile()
    return nc


def _strip_self_waits(nc):
    """Drop waits on an instruction's own engine semaphore.

    PE/ACT/DVE execute and complete their streams in order, so a wait on the
    engine's own proc sem is already enforced by program order.  Tile's
    vector clock still emits them (it is not transitively minimal), and the
    walrus MATMULT struct only fits one wait + one update — the redundant
    self-wait breaks codegen.  DMA lane FIFO is NOT guaranteed across HW
    queues and GpSimd runs 8 Q7s concurrently, so only PE/ACT/DVE non-DMA
    instructions are touched.
    """
    own = {"PE": "PE_", "Activation": "Activation_", "DVE": "DVE_"}
    for blk in nc.m.functions[0].blocks:
        for inst in blk.instructions:
            eng = getattr(inst, "engine", None)
            pref = own.get(getattr(eng, "name", None))
            if pref is None or "DMA" in inst.__class__.__name__:
                continue
            si = getattr(inst, "sync_info", None)
            if si and si.on_wait:
                kept = [
                    w
                    for w in si.on_wait
                    if not (getattr(w, "ant_name", "") or "").startswith(pref)
                ]
                if len(kept) != len(si.on_wait):
                    si.on_wait = kept


def _get_nc():
    if "nc" not in _cached:
        _cached["nc"] = _build()
    return _cached["nc"]


def _prep_inputs(x: np.ndarray, sigma: np.ndarray):
    s = abs(float(np.asarray(sigma).reshape(-1)[0])) + 1e-6
    r = np.arange(KS, dtype=np.float64) - (KS - 1) / 2.0
    g = np.exp(-(r**2) / (2.0 * s**2))
    g = g / g.sum()  # normalized 1D; outer(g, g) == reference 2D kernel
    A = _conv_matrix(g, H)  # (512, 512) float64, exact
    bf = ml_dtypes.bfloat16
    # device layouts are partition-major: [p][k][...] with h = 128*k + p
    at = A.T.reshape(NB, 128, H).transpose(1, 0, 2).astype(bf)

    xb = np.asarray(x, dtype=np.float32).reshape(N_CORES, PLANES, NB, 128, W)
    xb = xb.transpose(0, 1, 3, 2, 4).astype(bf)  # permute + cast in one pass
    return xb, at


def run(x: np.ndarray, sigma: np.ndarray, trace: bool = False):
    """Run on HW; returns (output fp32 (B,C,H,W), BassKernelResults)."""
    from concourse.bass_utils import run_bass_kernel_spmd

    nc = _get_nc()
    xb, at = _prep_inputs(x, sigma)
    in_maps = [{"x": xb[i], "at": at} for i in range(N_CORES)]
    res = run_bass_kernel_spmd(nc, in_maps, list(range(N_CORES)), trace=trace)
    out = np.empty((N_CORES, PLANES, H, W), dtype=np.float32)
    for i in range(N_CORES):
        yd = res.results[i]["y"]  # [PLANES, 128, NB, W], i_row = 128*k + p
        out[i] = yd.transpose(0, 2, 1, 3).reshape(PLANES, H, W).astype(np.float32)
    return out.reshape(B, C, H, W), res


def kernel(x: np.ndarray, sigma: np.ndarray) -> np.ndarray:
    out, _ = run(x, sigma, trace=False)
    return out
